# revision 9
# baseline (speedup 1.0000x reference)
"""Trainium2 kernel for the fused gather-transform problem.

Computes, for N points with per-point link ids and a tiny per-link [4,4]
homogeneous transform table:

    out[n] = R[link_ids[n]] @ pts[n] + t[link_ids[n]]

Sharding strategy: data-parallel over 8 cores (contiguous N/8 chunks).
Within each core's shard, the host groups points by link id (a counting
sort) and packs them into fixed-width single-link rows of W points.  Each
row carries its link's 12 transform scalars in a small side tensor, so the
device never performs a gather: every tile is processed with per-partition
scalar constants on the Scalar and Vector engines:

    out_i = (R_i2 * z) + ((R_i1 * y) + (R_i0 * x + t_i))

The device streams 2x ~13.4 MB per core (in + out); link ids never travel
to the device, so total HBM traffic is below the naive
pts+ids+out roofline.  The instruction structure is input-independent
(fixed tile count T); the row packing always fits for near-uniform link
distributions (T=4) and a T=5 variant provably fits any distribution.
"""

import math

import numpy as np

import concourse.bass as bass
import concourse.mybir as mybir
from concourse.tile import TileContext
from concourse.bass_utils import run_bass_kernel_spmd

N_PTS = 8388608
N_LINKS = 64
N_CORES = 8
NPC = N_PTS // N_CORES  # points per core
W = 2176                # points per packed row
ROWS_PER_TILE = 128


def _split_multiwait_drains(nc, max_waits=1):
    """walrus' codegen rejects instructions carrying more than a couple of
    semaphore waits; hoist extras onto preceding single-wait NoOps on the
    same engine (identical semantics on the serial engine stream)."""
    fn = nc.m.functions[0]
    for b in fn.blocks:
        newlist = []
        changed = False
        for i in b.instructions:
            si = i.sync_info
            if si is not None and len(si.on_wait) > max_waits:
                waits = list(si.on_wait)
                extra, keep = waits[:-max_waits], waits[-max_waits:]
                for k, w in enumerate(extra):
                    nop = mybir.InstNoOp(name=f"{i.name}-wsplit{k}", ins=[], outs=[])
                    nop.engine = i.engine
                    nop.sync_info = mybir.SyncInfo(on_wait=[w], on_update=[])
                    newlist.append(nop)
                i.sync_info = mybir.SyncInfo(on_wait=list(keep), on_update=list(si.on_update))
                changed = True
            newlist.append(i)
        if changed:
            b.instructions = newlist


def _build(T, repeat=1, hw_loop=0):
    f32 = mybir.dt.float32
    FD = 3 * W
    nc = bass.Bass(trn_type="TRN2")
    a = nc.dram_tensor("a", [T * ROWS_PER_TILE, FD], f32, kind="ExternalInput")
    s = nc.dram_tensor("s", [T * ROWS_PER_TILE, 12], f32, kind="ExternalInput")
    o = nc.dram_tensor("o", [T * ROWS_PER_TILE, FD], f32, kind="ExternalOutput")

    with TileContext(nc) as tc:
        with (
            tc.tile_pool(name="io", bufs=2) as iop,
            tc.tile_pool(name="acc", bufs=2) as accp,
            tc.tile_pool(name="scl", bufs=2) as sclp,
        ):
            def body():
                for _ in range(repeat):
                    for t in range(T):
                        r0 = t * ROWS_PER_TILE
                        tin = iop.tile([128, FD], f32, tag="tin")
                        nc.sync.dma_start(tin[:, :], a[r0:r0 + 128, :])
                        scl = sclp.tile([128, 12], f32, tag="scl")
                        nc.sync.dma_start(scl[:, :], s[r0:r0 + 128, :])
                        tout = iop.tile([128, FD], f32, tag="tout")
                        x = tin[:, 0:FD:3]
                        y = tin[:, 1:FD:3]
                        z = tin[:, 2:FD:3]
                        for i in range(3):
                            a1 = accp.tile([128, W], f32, tag="a1")
                            nc.scalar.activation(
                                a1[:, :], x,
                                mybir.ActivationFunctionType.Identity,
                                bias=scl[:, 9 + i:10 + i],
                                scale=scl[:, 3 * i:3 * i + 1],
                            )
                            a2 = accp.tile([128, W], f32, tag="a2")
                            nc.vector.scalar_tensor_tensor(
                                a2[:, :], y, scl[:, 3 * i + 1:3 * i + 2], a1[:, :],
                                mybir.AluOpType.mult, mybir.AluOpType.add,
                            )
                            nc.vector.scalar_tensor_tensor(
                                tout[:, i:FD:3], z, scl[:, 3 * i + 2:3 * i + 3], a2[:, :],
                                mybir.AluOpType.mult, mybir.AluOpType.add,
                            )
                        nc.sync.dma_start(o[r0:r0 + 128, :], tout[:, :])

            if hw_loop:
                with tc.For_i(0, hw_loop, 1):
                    body()
            else:
                body()

    _split_multiwait_drains(nc)
    return nc


_NC_CACHE = {}


def _get_nc(T, repeat=1, hw_loop=0):
    key = (T, repeat, hw_loop)
    if key not in _NC_CACHE:
        _NC_CACHE[key] = _build(T, repeat, hw_loop)
    return _NC_CACHE[key]


_EXEC_CACHE = {}


def _run_cached(nc, key, in_maps):
    """Like bass2jax.run_bass_via_pjrt, but caches the jitted executable so
    repeated kernel() calls don't recompile."""
    import jax
    from jax.sharding import Mesh, PartitionSpec
    from jax.experimental.shard_map import shard_map
    from concourse import bass2jax

    n_cores = len(in_maps)
    entry = _EXEC_CACHE.get(key)
    if entry is None:
        bass2jax.install_neuronx_cc_hook()
        partition_name = (
            nc.partition_id_tensor.name if nc.partition_id_tensor else None
        )
        in_names, out_names, out_avals = [], [], []
        for alloc in nc.m.functions[0].allocations:
            if not isinstance(alloc, mybir.MemoryLocationSet):
                continue
            name = alloc.memorylocations[0].name
            if alloc.kind == "ExternalInput":
                if name != partition_name:
                    in_names.append(name)
            elif alloc.kind == "ExternalOutput":
                out_names.append(name)
                shape = tuple(alloc.tensor_shape)
                out_avals.append(
                    jax.core.ShapedArray(shape, mybir.dt.np(alloc.dtype))
                )
        n_params = len(in_names)
        all_in_names = list(in_names) + list(out_names)
        if partition_name is not None:
            all_in_names.append(partition_name)
        donate = tuple(range(n_params, n_params + len(out_names)))

        def _body(*args):
            operands = list(args)
            if partition_name is not None:
                operands.append(bass2jax.partition_id_tensor())
            outs = bass2jax._bass_exec_p.bind(
                *operands,
                out_avals=tuple(out_avals),
                in_names=tuple(all_in_names),
                out_names=tuple(out_names),
                lowering_input_output_aliases=(),
                sim_require_finite=True,
                sim_require_nnan=True,
                nc=nc,
            )
            return tuple(outs)

        devices = jax.devices()[:n_cores]
        mesh = Mesh(np.asarray(devices), ("core",))
        in_specs = (PartitionSpec("core"),) * (n_params + len(out_names))
        out_specs = (PartitionSpec("core"),) * len(out_names)
        sharded = jax.jit(
            shard_map(_body, mesh=mesh, in_specs=in_specs,
                      out_specs=out_specs, check_rep=False),
            donate_argnums=donate, keep_unused=True,
        )
        entry = (sharded, in_names, out_names, out_avals)
        _EXEC_CACHE[key] = entry

    sharded, in_names, out_names, out_avals = entry
    concat_in = [
        np.concatenate([m[name] for m in in_maps], axis=0) for name in in_names
    ]
    concat_zeros = [
        np.zeros((n_cores * av.shape[0], *av.shape[1:]), av.dtype)
        for av in out_avals
    ]
    out_arrs = sharded(*concat_in, *concat_zeros)
    return [
        {
            name: np.asarray(out_arrs[i]).reshape(n_cores, *out_avals[i].shape)[c]
            for i, name in enumerate(out_names)
        }
        for c in range(n_cores)
    ]


def _pack_core(pts_c, ids_c, M12, T):
    """Sort one core's points by link and pack into single-link rows.

    Returns (A [T*128, 3W] f32, S [T*128, 12] f32, order, cnt, rowstart)
    or None if the packing does not fit in T tiles.
    """
    order = np.argsort(ids_c, kind="stable")
    sp = np.ascontiguousarray(pts_c[order])
    cnt = np.bincount(ids_c, minlength=N_LINKS).astype(np.int64)
    nrows = (cnt + (W - 1)) // W
    if int(nrows.sum()) > T * ROWS_PER_TILE:
        return None
    rowstart = np.zeros(N_LINKS + 1, dtype=np.int64)
    np.cumsum(nrows, out=rowstart[1:])
    A = np.zeros((T * ROWS_PER_TILE, 3 * W), dtype=np.float32)
    S = np.zeros((T * ROWS_PER_TILE, 12), dtype=np.float32)
    off = np.zeros(N_LINKS + 1, dtype=np.int64)
    np.cumsum(cnt, out=off[1:])
    for l in range(N_LINKS):
        c = int(cnt[l])
        if c == 0:
            continue
        r0, r1 = int(rowstart[l]), int(rowstart[l + 1])
        dst = A[r0:r1].reshape(-1, 3)
        dst[:c] = sp[off[l]:off[l] + c]
        S[r0:r1] = M12[l]
    return A, S, order, cnt, rowstart


def _unpack_core(B, order, cnt, rowstart, out_c):
    off = 0
    sorted_out = np.empty((order.size, 3), dtype=np.float32)
    for l in range(N_LINKS):
        c = int(cnt[l])
        if c == 0:
            continue
        r0, r1 = int(rowstart[l]), int(rowstart[l + 1])
        sorted_out[off:off + c] = B[r0:r1].reshape(-1, 3)[:c]
        off += c
    out_c[order] = sorted_out


def kernel(pts, transforms, link_ids):
    pts = np.asarray(pts, dtype=np.float32)
    transforms = np.asarray(transforms, dtype=np.float32)
    link_ids_in = np.asarray(link_ids)
    ids = link_ids_in.astype(np.int64, copy=False)

    # Per-link 12 scalars: rows of R (i-major) then t.
    R = transforms[:, :3, :3].reshape(N_LINKS, 9)
    tvec = transforms[:, :3, 3]
    M12 = np.concatenate([R, tvec], axis=1).astype(np.float32)

    for T in (4, 5, 8, 16, 32):
        packs = []
        ok = True
        for c in range(N_CORES):
            p = _pack_core(
                pts[c * NPC:(c + 1) * NPC],
                ids[c * NPC:(c + 1) * NPC],
                M12, T,
            )
            if p is None:
                ok = False
                break
            packs.append(p)
        if ok:
            break
    assert ok, "row packing failed at all tile counts"

    nc = _get_nc(T)
    in_maps = [{"a": p[0], "s": p[1]} for p in packs]
    try:
        results = _run_cached(nc, ("main", T), in_maps)
    except Exception:
        res = run_bass_kernel_spmd(nc, in_maps, core_ids=list(range(N_CORES)))
        results = res.results

    out = np.empty((N_PTS, 3), dtype=np.float32)
    for c in range(N_CORES):
        _, _, order, cnt, rowstart = packs[c]
        _unpack_core(
            results[c]["o"], order, cnt, rowstart,
            out[c * NPC:(c + 1) * NPC],
        )
    return out, link_ids_in


# revision 11
# speedup vs baseline: 5.8554x; 5.8554x over previous
"""Trainium2 kernel for the fused gather-transform problem.

Computes, for N points with per-point link ids and a tiny per-link [4,4]
homogeneous transform table:

    out[n] = R[link_ids[n]] @ pts[n] + t[link_ids[n]]

Sharding strategy: data-parallel over 8 cores (contiguous N/8 chunks).
Within each core's shard, the host groups points by link id (a counting
sort) and packs them into fixed-width single-link rows of W points.  Each
row carries its link's 12 transform scalars in a small side tensor, so the
device never performs a gather: every tile is processed with per-partition
scalar constants on the Scalar and Vector engines:

    out_i = (R_i2 * z) + ((R_i1 * y) + (R_i0 * x + t_i))

The device streams 2x ~13.4 MB per core (in + out); link ids never travel
to the device, so total HBM traffic is below the naive
pts+ids+out roofline.  The instruction structure is input-independent
(fixed tile count T); the row packing always fits for near-uniform link
distributions (T=4) and a T=5 variant provably fits any distribution.
"""

import math

import numpy as np

import concourse.bass as bass
import concourse.mybir as mybir
from concourse.tile import TileContext
from concourse.bass_utils import run_bass_kernel_spmd

N_PTS = 8388608
N_LINKS = 64
N_CORES = 8
NPC = N_PTS // N_CORES  # points per core
W = 2176                # points per packed row
ROWS_PER_TILE = 128


def _split_multiwait_drains(nc, max_waits=1):
    """walrus' codegen rejects instructions carrying more than a couple of
    semaphore waits; hoist extras onto preceding single-wait NoOps on the
    same engine (identical semantics on the serial engine stream)."""
    fn = nc.m.functions[0]
    for b in fn.blocks:
        newlist = []
        changed = False
        for i in b.instructions:
            si = i.sync_info
            if si is not None and len(si.on_wait) > max_waits:
                waits = list(si.on_wait)
                extra, keep = waits[:-max_waits], waits[-max_waits:]
                for k, w in enumerate(extra):
                    nop = mybir.InstNoOp(name=f"{i.name}-wsplit{k}", ins=[], outs=[])
                    nop.engine = i.engine
                    nop.sync_info = mybir.SyncInfo(on_wait=[w], on_update=[])
                    newlist.append(nop)
                i.sync_info = mybir.SyncInfo(on_wait=list(keep), on_update=list(si.on_update))
                changed = True
            newlist.append(i)
        if changed:
            b.instructions = newlist


def _build(T, hw_loop=0, wc=1088, bufs_in=4, bufs_out=3, gps=0):
    """wc: points per device chunk (must divide W); gps: how many of the
    three y-accumulate ops run on GPSIMD instead of DVE."""
    f32 = mybir.dt.float32
    FD = 3 * W
    FC = 3 * wc
    assert W % wc == 0
    nc = bass.Bass(trn_type="TRN2")
    a = nc.dram_tensor("a", [T * ROWS_PER_TILE, FD], f32, kind="ExternalInput")
    s = nc.dram_tensor("s", [T * ROWS_PER_TILE, 12], f32, kind="ExternalInput")
    o = nc.dram_tensor("o", [T * ROWS_PER_TILE, FD], f32, kind="ExternalOutput")

    with TileContext(nc) as tc:
        with (
            tc.tile_pool(name="tin", bufs=bufs_in) as inp,
            tc.tile_pool(name="tout", bufs=bufs_out) as outp,
            tc.tile_pool(name="acc", bufs=2) as accp,
            tc.tile_pool(name="scl", bufs=1) as sclp,
        ):
            # Preload all per-row transform scalars once: scl[p, 12t + c]
            # = s[t*128 + p, c].
            scl = sclp.tile([128, 12 * T], f32, tag="scl")
            s_ap = bass.AP(s, 0, [[12, 128], [12 * ROWS_PER_TILE, T], [1, 12]])
            nc.sync.dma_start(scl[:, :], s_ap)

            def body():
                for t in range(T):
                    r0 = t * ROWS_PER_TILE
                    sc = 12 * t
                    for cb in range(W // wc):
                        c0 = cb * FC
                        tin = inp.tile([128, FC], f32, tag="tin")
                        nc.sync.dma_start(tin[:, :], a[r0:r0 + 128, c0:c0 + FC])
                        tout = outp.tile([128, FC], f32, tag="tout")
                        x = tin[:, 0:FC:3]
                        y = tin[:, 1:FC:3]
                        z = tin[:, 2:FC:3]
                        for i in range(3):
                            a1 = accp.tile([128, wc], f32, tag="a1")
                            nc.scalar.activation(
                                a1[:, :], x,
                                mybir.ActivationFunctionType.Identity,
                                bias=scl[:, sc + 9 + i:sc + 10 + i],
                                scale=scl[:, sc + 3 * i:sc + 3 * i + 1],
                            )
                            a2 = accp.tile([128, wc], f32, tag="a2")
                            eng = nc.gpsimd if i < gps else nc.vector
                            eng.scalar_tensor_tensor(
                                a2[:, :], y, scl[:, sc + 3 * i + 1:sc + 3 * i + 2], a1[:, :],
                                mybir.AluOpType.mult, mybir.AluOpType.add,
                            )
                            nc.vector.scalar_tensor_tensor(
                                tout[:, i:FC:3], z, scl[:, sc + 3 * i + 2:sc + 3 * i + 3], a2[:, :],
                                mybir.AluOpType.mult, mybir.AluOpType.add,
                            )
                        nc.sync.dma_start(o[r0:r0 + 128, c0:c0 + FC], tout[:, :])

            if hw_loop:
                with tc.For_i(0, hw_loop, 1):
                    body()
            else:
                body()

    _split_multiwait_drains(nc)
    return nc


_NC_CACHE = {}


def _get_nc(T, hw_loop=0, **kw):
    key = (T, hw_loop, tuple(sorted(kw.items())))
    if key not in _NC_CACHE:
        _NC_CACHE[key] = _build(T, hw_loop, **kw)
    return _NC_CACHE[key]


_EXEC_CACHE = {}


def _run_cached(nc, key, in_maps):
    """Like bass2jax.run_bass_via_pjrt, but caches the jitted executable so
    repeated kernel() calls don't recompile."""
    import jax
    from jax.sharding import Mesh, PartitionSpec
    from jax.experimental.shard_map import shard_map
    from concourse import bass2jax

    n_cores = len(in_maps)
    entry = _EXEC_CACHE.get(key)
    if entry is None:
        bass2jax.install_neuronx_cc_hook()
        partition_name = (
            nc.partition_id_tensor.name if nc.partition_id_tensor else None
        )
        in_names, out_names, out_avals = [], [], []
        for alloc in nc.m.functions[0].allocations:
            if not isinstance(alloc, mybir.MemoryLocationSet):
                continue
            name = alloc.memorylocations[0].name
            if alloc.kind == "ExternalInput":
                if name != partition_name:
                    in_names.append(name)
            elif alloc.kind == "ExternalOutput":
                out_names.append(name)
                shape = tuple(alloc.tensor_shape)
                out_avals.append(
                    jax.core.ShapedArray(shape, mybir.dt.np(alloc.dtype))
                )
        n_params = len(in_names)
        all_in_names = list(in_names) + list(out_names)
        if partition_name is not None:
            all_in_names.append(partition_name)
        donate = tuple(range(n_params, n_params + len(out_names)))

        def _body(*args):
            operands = list(args)
            if partition_name is not None:
                operands.append(bass2jax.partition_id_tensor())
            outs = bass2jax._bass_exec_p.bind(
                *operands,
                out_avals=tuple(out_avals),
                in_names=tuple(all_in_names),
                out_names=tuple(out_names),
                lowering_input_output_aliases=(),
                sim_require_finite=True,
                sim_require_nnan=True,
                nc=nc,
            )
            return tuple(outs)

        devices = jax.devices()[:n_cores]
        mesh = Mesh(np.asarray(devices), ("core",))
        in_specs = (PartitionSpec("core"),) * (n_params + len(out_names))
        out_specs = (PartitionSpec("core"),) * len(out_names)
        sharded = jax.jit(
            shard_map(_body, mesh=mesh, in_specs=in_specs,
                      out_specs=out_specs, check_rep=False),
            donate_argnums=donate, keep_unused=True,
        )
        entry = (sharded, in_names, out_names, out_avals)
        _EXEC_CACHE[key] = entry

    sharded, in_names, out_names, out_avals = entry
    concat_in = [
        np.concatenate([m[name] for m in in_maps], axis=0) for name in in_names
    ]
    concat_zeros = [
        np.zeros((n_cores * av.shape[0], *av.shape[1:]), av.dtype)
        for av in out_avals
    ]
    out_arrs = sharded(*concat_in, *concat_zeros)
    return [
        {
            name: np.asarray(out_arrs[i]).reshape(n_cores, *out_avals[i].shape)[c]
            for i, name in enumerate(out_names)
        }
        for c in range(n_cores)
    ]


def _pack_core(pts_c, ids_c, M12, T):
    """Sort one core's points by link and pack into single-link rows.

    Returns (A [T*128, 3W] f32, S [T*128, 12] f32, order, cnt, rowstart)
    or None if the packing does not fit in T tiles.
    """
    order = np.argsort(ids_c, kind="stable")
    sp = np.ascontiguousarray(pts_c[order])
    cnt = np.bincount(ids_c, minlength=N_LINKS).astype(np.int64)
    nrows = (cnt + (W - 1)) // W
    if int(nrows.sum()) > T * ROWS_PER_TILE:
        return None
    rowstart = np.zeros(N_LINKS + 1, dtype=np.int64)
    np.cumsum(nrows, out=rowstart[1:])
    A = np.zeros((T * ROWS_PER_TILE, 3 * W), dtype=np.float32)
    S = np.zeros((T * ROWS_PER_TILE, 12), dtype=np.float32)
    off = np.zeros(N_LINKS + 1, dtype=np.int64)
    np.cumsum(cnt, out=off[1:])
    for l in range(N_LINKS):
        c = int(cnt[l])
        if c == 0:
            continue
        r0, r1 = int(rowstart[l]), int(rowstart[l + 1])
        dst = A[r0:r1].reshape(-1, 3)
        dst[:c] = sp[off[l]:off[l] + c]
        S[r0:r1] = M12[l]
    return A, S, order, cnt, rowstart


def _unpack_core(B, order, cnt, rowstart, out_c):
    off = 0
    sorted_out = np.empty((order.size, 3), dtype=np.float32)
    for l in range(N_LINKS):
        c = int(cnt[l])
        if c == 0:
            continue
        r0, r1 = int(rowstart[l]), int(rowstart[l + 1])
        sorted_out[off:off + c] = B[r0:r1].reshape(-1, 3)[:c]
        off += c
    out_c[order] = sorted_out


def kernel(pts, transforms, link_ids):
    pts = np.asarray(pts, dtype=np.float32)
    transforms = np.asarray(transforms, dtype=np.float32)
    link_ids_in = np.asarray(link_ids)
    ids = link_ids_in.astype(np.int64, copy=False)

    # Per-link 12 scalars: rows of R (i-major) then t.
    R = transforms[:, :3, :3].reshape(N_LINKS, 9)
    tvec = transforms[:, :3, 3]
    M12 = np.concatenate([R, tvec], axis=1).astype(np.float32)

    for T in (4, 5, 8, 16, 32):
        packs = []
        ok = True
        for c in range(N_CORES):
            p = _pack_core(
                pts[c * NPC:(c + 1) * NPC],
                ids[c * NPC:(c + 1) * NPC],
                M12, T,
            )
            if p is None:
                ok = False
                break
            packs.append(p)
        if ok:
            break
    assert ok, "row packing failed at all tile counts"

    nc = _get_nc(T)
    in_maps = [{"a": p[0], "s": p[1]} for p in packs]
    try:
        results = _run_cached(nc, ("main", T), in_maps)
    except Exception:
        res = run_bass_kernel_spmd(nc, in_maps, core_ids=list(range(N_CORES)))
        results = res.results

    out = np.empty((N_PTS, 3), dtype=np.float32)
    for c in range(N_CORES):
        _, _, order, cnt, rowstart = packs[c]
        _unpack_core(
            results[c]["o"], order, cnt, rowstart,
            out[c * NPC:(c + 1) * NPC],
        )
    return out, link_ids_in


# revision 12
# speedup vs baseline: 6.8850x; 1.1758x over previous
"""Trainium2 kernel for the fused gather-transform problem.

Computes, for N points with per-point link ids and a tiny per-link [4,4]
homogeneous transform table:

    out[n] = R[link_ids[n]] @ pts[n] + t[link_ids[n]]

Sharding strategy: data-parallel over 8 cores (contiguous N/8 chunks).
Within each core's shard, the host groups points by link id (a counting
sort) and packs them into fixed-width single-link rows of W points.  Each
row carries its link's 12 transform scalars in a small side tensor, so the
device never performs a gather: every tile is processed with per-partition
scalar constants on the Scalar and Vector engines:

    out_i = (R_i2 * z) + ((R_i1 * y) + (R_i0 * x + t_i))

The device streams 2x ~13.4 MB per core (in + out); link ids never travel
to the device, so total HBM traffic is below the naive
pts+ids+out roofline.  The instruction structure is input-independent
(fixed tile count T); the row packing always fits for near-uniform link
distributions (T=4) and a T=5 variant provably fits any distribution.
"""

import numpy as np

import concourse.bass as bass
import concourse.mybir as mybir
from concourse.tile import TileContext
from concourse.bass_utils import run_bass_kernel_spmd

N_PTS = 8388608
N_LINKS = 64
N_CORES = 8
NPC = N_PTS // N_CORES  # points per core
W = 2176                # points per packed row
ROWS_PER_TILE = 128


def _split_multiwait_drains(nc, max_waits=1):
    """walrus' codegen rejects instructions carrying more than a couple of
    semaphore waits; hoist extras onto preceding single-wait NoOps on the
    same engine (identical semantics on the serial engine stream)."""
    fn = nc.m.functions[0]
    for b in fn.blocks:
        newlist = []
        changed = False
        for i in b.instructions:
            si = i.sync_info
            if si is not None and len(si.on_wait) > max_waits:
                waits = list(si.on_wait)
                extra, keep = waits[:-max_waits], waits[-max_waits:]
                for k, w in enumerate(extra):
                    nop = mybir.InstNoOp(name=f"{i.name}-wsplit{k}", ins=[], outs=[])
                    nop.engine = i.engine
                    nop.sync_info = mybir.SyncInfo(on_wait=[w], on_update=[])
                    newlist.append(nop)
                i.sync_info = mybir.SyncInfo(on_wait=list(keep), on_update=list(si.on_update))
                changed = True
            newlist.append(i)
        if changed:
            b.instructions = newlist


def _build(T, hw_loop=0, wc=1088, bufs_in=4, bufs_out=3, gps=0):
    """wc: points per device chunk (must divide W); gps: how many of the
    three y-accumulate ops run on GPSIMD instead of DVE."""
    f32 = mybir.dt.float32
    FD = 3 * W
    FC = 3 * wc
    assert W % wc == 0
    nc = bass.Bass(trn_type="TRN2")
    a = nc.dram_tensor("a", [T * ROWS_PER_TILE, FD], f32, kind="ExternalInput")
    s = nc.dram_tensor("s", [T * ROWS_PER_TILE, 12], f32, kind="ExternalInput")
    o = nc.dram_tensor("o", [T * ROWS_PER_TILE, FD], f32, kind="ExternalOutput")

    with TileContext(nc) as tc:
        with (
            tc.tile_pool(name="tin", bufs=bufs_in) as inp,
            tc.tile_pool(name="tout", bufs=bufs_out) as outp,
            tc.tile_pool(name="acc", bufs=2) as accp,
            tc.tile_pool(name="scl", bufs=1) as sclp,
        ):
            # Preload all per-row transform scalars once: scl[p, 12t + c]
            # = s[t*128 + p, c].
            scl = sclp.tile([128, 12 * T], f32, tag="scl")
            s_ap = bass.AP(s, 0, [[12, 128], [12 * ROWS_PER_TILE, T], [1, 12]])
            nc.sync.dma_start(scl[:, :], s_ap)

            def body():
                for t in range(T):
                    r0 = t * ROWS_PER_TILE
                    sc = 12 * t
                    for cb in range(W // wc):
                        c0 = cb * FC
                        tin = inp.tile([128, FC], f32, tag="tin")
                        nc.sync.dma_start(tin[:, :], a[r0:r0 + 128, c0:c0 + FC])
                        tout = outp.tile([128, FC], f32, tag="tout")
                        x = tin[:, 0:FC:3]
                        y = tin[:, 1:FC:3]
                        z = tin[:, 2:FC:3]
                        for i in range(3):
                            a1 = accp.tile([128, wc], f32, tag="a1")
                            nc.scalar.activation(
                                a1[:, :], x,
                                mybir.ActivationFunctionType.Identity,
                                bias=scl[:, sc + 9 + i:sc + 10 + i],
                                scale=scl[:, sc + 3 * i:sc + 3 * i + 1],
                            )
                            a2 = accp.tile([128, wc], f32, tag="a2")
                            eng = nc.gpsimd if i < gps else nc.vector
                            eng.scalar_tensor_tensor(
                                a2[:, :], y, scl[:, sc + 3 * i + 1:sc + 3 * i + 2], a1[:, :],
                                mybir.AluOpType.mult, mybir.AluOpType.add,
                            )
                            nc.vector.scalar_tensor_tensor(
                                tout[:, i:FC:3], z, scl[:, sc + 3 * i + 2:sc + 3 * i + 3], a2[:, :],
                                mybir.AluOpType.mult, mybir.AluOpType.add,
                            )
                        nc.sync.dma_start(o[r0:r0 + 128, c0:c0 + FC], tout[:, :])

            if hw_loop:
                with tc.For_i(0, hw_loop, 1):
                    body()
            else:
                body()

    _split_multiwait_drains(nc)
    return nc


_NC_CACHE = {}


def _get_nc(T, hw_loop=0, **kw):
    key = (T, hw_loop, tuple(sorted(kw.items())))
    if key not in _NC_CACHE:
        _NC_CACHE[key] = _build(T, hw_loop, **kw)
    return _NC_CACHE[key]


_EXEC_CACHE = {}


def _run_cached(nc, key, in_maps):
    """Like bass2jax.run_bass_via_pjrt, but caches the jitted executable so
    repeated kernel() calls don't recompile."""
    import jax
    from jax.sharding import Mesh, PartitionSpec
    from jax.experimental.shard_map import shard_map
    from concourse import bass2jax

    n_cores = len(in_maps)
    entry = _EXEC_CACHE.get(key)
    if entry is None:
        bass2jax.install_neuronx_cc_hook()
        partition_name = (
            nc.partition_id_tensor.name if nc.partition_id_tensor else None
        )
        in_names, out_names, out_avals = [], [], []
        for alloc in nc.m.functions[0].allocations:
            if not isinstance(alloc, mybir.MemoryLocationSet):
                continue
            name = alloc.memorylocations[0].name
            if alloc.kind == "ExternalInput":
                if name != partition_name:
                    in_names.append(name)
            elif alloc.kind == "ExternalOutput":
                out_names.append(name)
                shape = tuple(alloc.tensor_shape)
                out_avals.append(
                    jax.core.ShapedArray(shape, mybir.dt.np(alloc.dtype))
                )
        n_params = len(in_names)
        all_in_names = list(in_names) + list(out_names)
        if partition_name is not None:
            all_in_names.append(partition_name)
        donate = tuple(range(n_params, n_params + len(out_names)))

        def _body(*args):
            operands = list(args)
            if partition_name is not None:
                operands.append(bass2jax.partition_id_tensor())
            outs = bass2jax._bass_exec_p.bind(
                *operands,
                out_avals=tuple(out_avals),
                in_names=tuple(all_in_names),
                out_names=tuple(out_names),
                lowering_input_output_aliases=(),
                sim_require_finite=True,
                sim_require_nnan=True,
                nc=nc,
            )
            return tuple(outs)

        devices = jax.devices()[:n_cores]
        mesh = Mesh(np.asarray(devices), ("core",))
        in_specs = (PartitionSpec("core"),) * (n_params + len(out_names))
        out_specs = (PartitionSpec("core"),) * len(out_names)
        sharded = jax.jit(
            shard_map(_body, mesh=mesh, in_specs=in_specs,
                      out_specs=out_specs, check_rep=False),
            donate_argnums=donate, keep_unused=True,
        )
        entry = (sharded, in_names, out_names, out_avals)
        _EXEC_CACHE[key] = entry

    sharded, in_names, out_names, out_avals = entry
    concat_in = [
        np.concatenate([m[name] for m in in_maps], axis=0) for name in in_names
    ]
    concat_zeros = [
        np.zeros((n_cores * av.shape[0], *av.shape[1:]), av.dtype)
        for av in out_avals
    ]
    out_arrs = sharded(*concat_in, *concat_zeros)
    return [
        {
            name: np.asarray(out_arrs[i]).reshape(n_cores, *out_avals[i].shape)[c]
            for i, name in enumerate(out_names)
        }
        for c in range(n_cores)
    ]


def _pack_core(pts_c, ids_c, M12, T):
    """Sort one core's points by link and pack into single-link rows.

    Returns (A [T*128, 3W] f32, S [T*128, 12] f32, order, cnt, rowstart)
    or None if the packing does not fit in T tiles.
    """
    order = np.argsort(ids_c, kind="stable")
    sp = np.ascontiguousarray(pts_c[order])
    cnt = np.bincount(ids_c, minlength=N_LINKS).astype(np.int64)
    nrows = (cnt + (W - 1)) // W
    if int(nrows.sum()) > T * ROWS_PER_TILE:
        return None
    rowstart = np.zeros(N_LINKS + 1, dtype=np.int64)
    np.cumsum(nrows, out=rowstart[1:])
    A = np.zeros((T * ROWS_PER_TILE, 3 * W), dtype=np.float32)
    S = np.zeros((T * ROWS_PER_TILE, 12), dtype=np.float32)
    off = np.zeros(N_LINKS + 1, dtype=np.int64)
    np.cumsum(cnt, out=off[1:])
    for l in range(N_LINKS):
        c = int(cnt[l])
        if c == 0:
            continue
        r0, r1 = int(rowstart[l]), int(rowstart[l + 1])
        dst = A[r0:r1].reshape(-1, 3)
        dst[:c] = sp[off[l]:off[l] + c]
        S[r0:r1] = M12[l]
    return A, S, order, cnt, rowstart


def _unpack_core(B, order, cnt, rowstart, out_c):
    off = 0
    sorted_out = np.empty((order.size, 3), dtype=np.float32)
    for l in range(N_LINKS):
        c = int(cnt[l])
        if c == 0:
            continue
        r0, r1 = int(rowstart[l]), int(rowstart[l + 1])
        sorted_out[off:off + c] = B[r0:r1].reshape(-1, 3)[:c]
        off += c
    out_c[order] = sorted_out


def kernel(pts, transforms, link_ids):
    pts = np.asarray(pts, dtype=np.float32)
    transforms = np.asarray(transforms, dtype=np.float32)
    link_ids_in = np.asarray(link_ids)
    ids = link_ids_in.astype(np.int64, copy=False)

    # Per-link 12 scalars: rows of R (i-major) then t.
    R = transforms[:, :3, :3].reshape(N_LINKS, 9)
    tvec = transforms[:, :3, 3]
    M12 = np.concatenate([R, tvec], axis=1).astype(np.float32)

    for T in (4, 5, 8, 16, 32):
        packs = []
        ok = True
        for c in range(N_CORES):
            p = _pack_core(
                pts[c * NPC:(c + 1) * NPC],
                ids[c * NPC:(c + 1) * NPC],
                M12, T,
            )
            if p is None:
                ok = False
                break
            packs.append(p)
        if ok:
            break
    assert ok, "row packing failed at all tile counts"

    nc = _get_nc(T)
    in_maps = [{"a": p[0], "s": p[1]} for p in packs]
    try:
        results = _run_cached(nc, ("main", T), in_maps)
    except Exception:
        res = run_bass_kernel_spmd(nc, in_maps, core_ids=list(range(N_CORES)))
        results = res.results

    out = np.empty((N_PTS, 3), dtype=np.float32)
    for c in range(N_CORES):
        _, _, order, cnt, rowstart = packs[c]
        _unpack_core(
            results[c]["o"], order, cnt, rowstart,
            out[c * NPC:(c + 1) * NPC],
        )
    return out, link_ids_in


# revision 16
# speedup vs baseline: 7.0909x; 1.0299x over previous
"""Trainium2 kernel for the fused gather-transform problem.

Computes, for N points with per-point link ids and a tiny per-link [4,4]
homogeneous transform table:

    out[n] = R[link_ids[n]] @ pts[n] + t[link_ids[n]]

Sharding strategy: data-parallel over 8 cores (contiguous N/8 chunks).
Within each core's shard, the host groups points by link id (a counting
sort) and packs them into fixed-width single-link rows of W points.  Each
row carries its link's 12 transform scalars in a small side tensor, so the
device never performs a gather: every tile is processed with per-partition
scalar constants on the Scalar and Vector engines:

    out_i = (R_i2 * z) + ((R_i1 * y) + (R_i0 * x + t_i))

The device streams 2x ~13.4 MB per core (in + out); link ids never travel
to the device, so total HBM traffic is below the naive
pts+ids+out roofline.  The instruction structure is input-independent
(fixed tile count T); the row packing always fits for near-uniform link
distributions (T=4) and a T=5 variant provably fits any distribution.
"""

import numpy as np

import concourse.bass as bass
import concourse.mybir as mybir
from concourse.tile import TileContext
from concourse.bass_utils import run_bass_kernel_spmd

N_PTS = 8388608
N_LINKS = 64
N_CORES = 8
NPC = N_PTS // N_CORES  # points per core
W = 2176                # points per packed row
ROWS_PER_TILE = 128


def _split_multiwait_drains(nc, max_waits=1):
    """walrus' codegen rejects instructions carrying more than a couple of
    semaphore waits; hoist extras onto preceding single-wait NoOps on the
    same engine (identical semantics on the serial engine stream)."""
    fn = nc.m.functions[0]
    for b in fn.blocks:
        newlist = []
        changed = False
        for i in b.instructions:
            si = i.sync_info
            if si is not None and len(si.on_wait) > max_waits:
                waits = list(si.on_wait)
                extra, keep = waits[:-max_waits], waits[-max_waits:]
                for k, w in enumerate(extra):
                    nop = mybir.InstNoOp(name=f"{i.name}-wsplit{k}", ins=[], outs=[])
                    nop.engine = i.engine
                    nop.sync_info = mybir.SyncInfo(on_wait=[w], on_update=[])
                    newlist.append(nop)
                i.sync_info = mybir.SyncInfo(on_wait=list(keep), on_update=list(si.on_update))
                changed = True
            newlist.append(i)
        if changed:
            b.instructions = newlist


def _build(T, hw_loop=0, wc=1088, bufs_in=4, bufs_out=3, gps=0, dual=0):
    """wc: points per device chunk (must divide W); gps: how many of the
    three y-accumulate ops run on GPSIMD instead of DVE; dual: issue output
    DMAs on the second HWDGE ring (ACT sequencer) instead of SP."""
    f32 = mybir.dt.float32
    FD = 3 * W
    FC = 3 * wc
    assert W % wc == 0
    nc = bass.Bass(trn_type="TRN2")
    a = nc.dram_tensor("a", [T * ROWS_PER_TILE, FD], f32, kind="ExternalInput")
    s = nc.dram_tensor("s", [T * ROWS_PER_TILE, 12], f32, kind="ExternalInput")
    o = nc.dram_tensor("o", [T * ROWS_PER_TILE, FD], f32, kind="ExternalOutput")

    with TileContext(nc) as tc:
        with (
            tc.tile_pool(name="tin", bufs=bufs_in) as inp,
            tc.tile_pool(name="tout", bufs=bufs_out) as outp,
            tc.tile_pool(name="acc", bufs=2) as accp,
            tc.tile_pool(name="scl", bufs=1) as sclp,
        ):
            # Preload all per-row transform scalars once: scl[p, 12t + c]
            # = s[t*128 + p, c].
            scl = sclp.tile([128, 12 * T], f32, tag="scl")
            s_ap = bass.AP(s, 0, [[12, 128], [12 * ROWS_PER_TILE, T], [1, 12]])
            nc.sync.dma_start(scl[:, :], s_ap)

            def body():
                for t in range(T):
                    r0 = t * ROWS_PER_TILE
                    sc = 12 * t
                    for cb in range(W // wc):
                        c0 = cb * FC
                        tin = inp.tile([128, FC], f32, tag="tin")
                        nc.sync.dma_start(tin[:, :], a[r0:r0 + 128, c0:c0 + FC])
                        tout = outp.tile([128, FC], f32, tag="tout")
                        x = tin[:, 0:FC:3]
                        y = tin[:, 1:FC:3]
                        z = tin[:, 2:FC:3]
                        for i in range(3):
                            a1 = accp.tile([128, wc], f32, tag="a1")
                            nc.scalar.activation(
                                a1[:, :], x,
                                mybir.ActivationFunctionType.Identity,
                                bias=scl[:, sc + 9 + i:sc + 10 + i],
                                scale=scl[:, sc + 3 * i:sc + 3 * i + 1],
                            )
                            a2 = accp.tile([128, wc], f32, tag="a2")
                            eng = nc.gpsimd if i < gps else nc.vector
                            eng.scalar_tensor_tensor(
                                a2[:, :], y, scl[:, sc + 3 * i + 1:sc + 3 * i + 2], a1[:, :],
                                mybir.AluOpType.mult, mybir.AluOpType.add,
                            )
                            nc.vector.scalar_tensor_tensor(
                                tout[:, i:FC:3], z, scl[:, sc + 3 * i + 2:sc + 3 * i + 3], a2[:, :],
                                mybir.AluOpType.mult, mybir.AluOpType.add,
                            )
                        out_eng = nc.scalar if dual else nc.sync
                        out_eng.dma_start(o[r0:r0 + 128, c0:c0 + FC], tout[:, :])

            if hw_loop:
                with tc.For_i(0, hw_loop, 1):
                    body()
            else:
                body()

    _split_multiwait_drains(nc)
    return nc


_NC_CACHE = {}


def _get_nc(T, hw_loop=0, **kw):
    key = (T, hw_loop, tuple(sorted(kw.items())))
    if key not in _NC_CACHE:
        _NC_CACHE[key] = _build(T, hw_loop, **kw)
    return _NC_CACHE[key]


_EXEC_CACHE = {}


def _run_cached(nc, key, in_maps):
    """Like bass2jax.run_bass_via_pjrt, but caches the jitted executable so
    repeated kernel() calls don't recompile."""
    import jax
    from jax.sharding import Mesh, PartitionSpec
    from jax.experimental.shard_map import shard_map
    from concourse import bass2jax

    n_cores = len(in_maps)
    entry = _EXEC_CACHE.get(key)
    if entry is None:
        bass2jax.install_neuronx_cc_hook()
        partition_name = (
            nc.partition_id_tensor.name if nc.partition_id_tensor else None
        )
        in_names, out_names, out_avals = [], [], []
        for alloc in nc.m.functions[0].allocations:
            if not isinstance(alloc, mybir.MemoryLocationSet):
                continue
            name = alloc.memorylocations[0].name
            if alloc.kind == "ExternalInput":
                if name != partition_name:
                    in_names.append(name)
            elif alloc.kind == "ExternalOutput":
                out_names.append(name)
                shape = tuple(alloc.tensor_shape)
                out_avals.append(
                    jax.core.ShapedArray(shape, mybir.dt.np(alloc.dtype))
                )
        n_params = len(in_names)
        all_in_names = list(in_names) + list(out_names)
        if partition_name is not None:
            all_in_names.append(partition_name)
        donate = tuple(range(n_params, n_params + len(out_names)))

        def _body(*args):
            operands = list(args)
            if partition_name is not None:
                operands.append(bass2jax.partition_id_tensor())
            outs = bass2jax._bass_exec_p.bind(
                *operands,
                out_avals=tuple(out_avals),
                in_names=tuple(all_in_names),
                out_names=tuple(out_names),
                lowering_input_output_aliases=(),
                sim_require_finite=True,
                sim_require_nnan=True,
                nc=nc,
            )
            return tuple(outs)

        devices = jax.devices()[:n_cores]
        mesh = Mesh(np.asarray(devices), ("core",))
        in_specs = (PartitionSpec("core"),) * (n_params + len(out_names))
        out_specs = (PartitionSpec("core"),) * len(out_names)
        sharded = jax.jit(
            shard_map(_body, mesh=mesh, in_specs=in_specs,
                      out_specs=out_specs, check_rep=False),
            donate_argnums=donate, keep_unused=True,
        )
        entry = (sharded, in_names, out_names, out_avals)
        _EXEC_CACHE[key] = entry

    sharded, in_names, out_names, out_avals = entry
    concat_in = [
        np.concatenate([m[name] for m in in_maps], axis=0) for name in in_names
    ]
    concat_zeros = [
        np.zeros((n_cores * av.shape[0], *av.shape[1:]), av.dtype)
        for av in out_avals
    ]
    out_arrs = sharded(*concat_in, *concat_zeros)
    return [
        {
            name: np.asarray(out_arrs[i]).reshape(n_cores, *out_avals[i].shape)[c]
            for i, name in enumerate(out_names)
        }
        for c in range(n_cores)
    ]


def _pack_core(pts_c, ids_c, M12, T):
    """Sort one core's points by link and pack into single-link rows.

    Returns (A [T*128, 3W] f32, S [T*128, 12] f32, order, cnt, rowstart)
    or None if the packing does not fit in T tiles.
    """
    order = np.argsort(ids_c, kind="stable")
    sp = np.ascontiguousarray(pts_c[order])
    cnt = np.bincount(ids_c, minlength=N_LINKS).astype(np.int64)
    nrows = (cnt + (W - 1)) // W
    if int(nrows.sum()) > T * ROWS_PER_TILE:
        return None
    rowstart = np.zeros(N_LINKS + 1, dtype=np.int64)
    np.cumsum(nrows, out=rowstart[1:])
    A = np.zeros((T * ROWS_PER_TILE, 3 * W), dtype=np.float32)
    S = np.zeros((T * ROWS_PER_TILE, 12), dtype=np.float32)
    off = np.zeros(N_LINKS + 1, dtype=np.int64)
    np.cumsum(cnt, out=off[1:])
    for l in range(N_LINKS):
        c = int(cnt[l])
        if c == 0:
            continue
        r0, r1 = int(rowstart[l]), int(rowstart[l + 1])
        dst = A[r0:r1].reshape(-1, 3)
        dst[:c] = sp[off[l]:off[l] + c]
        S[r0:r1] = M12[l]
    return A, S, order, cnt, rowstart


def _unpack_core(B, order, cnt, rowstart, out_c):
    off = 0
    sorted_out = np.empty((order.size, 3), dtype=np.float32)
    for l in range(N_LINKS):
        c = int(cnt[l])
        if c == 0:
            continue
        r0, r1 = int(rowstart[l]), int(rowstart[l + 1])
        sorted_out[off:off + c] = B[r0:r1].reshape(-1, 3)[:c]
        off += c
    out_c[order] = sorted_out


def kernel(pts, transforms, link_ids):
    pts = np.asarray(pts, dtype=np.float32)
    transforms = np.asarray(transforms, dtype=np.float32)
    link_ids_in = np.asarray(link_ids)
    # keep the native integer dtype: argsort's radix path is faster on int32
    ids = link_ids_in

    # Per-link 12 scalars: rows of R (i-major) then t.
    R = transforms[:, :3, :3].reshape(N_LINKS, 9)
    tvec = transforms[:, :3, 3]
    M12 = np.concatenate([R, tvec], axis=1).astype(np.float32)

    for T in (4, 5, 8, 16, 32):
        packs = []
        ok = True
        for c in range(N_CORES):
            p = _pack_core(
                pts[c * NPC:(c + 1) * NPC],
                ids[c * NPC:(c + 1) * NPC],
                M12, T,
            )
            if p is None:
                ok = False
                break
            packs.append(p)
        if ok:
            break
    assert ok, "row packing failed at all tile counts"

    nc = _get_nc(T, dual=1)
    in_maps = [{"a": p[0], "s": p[1]} for p in packs]
    try:
        results = _run_cached(nc, ("main", T), in_maps)
    except Exception:
        res = run_bass_kernel_spmd(nc, in_maps, core_ids=list(range(N_CORES)))
        results = res.results

    out = np.empty((N_PTS, 3), dtype=np.float32)
    for c in range(N_CORES):
        _, _, order, cnt, rowstart = packs[c]
        _unpack_core(
            results[c]["o"], order, cnt, rowstart,
            out[c * NPC:(c + 1) * NPC],
        )
    return out, link_ids_in
